# revision 2
# baseline (speedup 1.0000x reference)
"""Cross-attention kernel for Trainium2, sharded over 8 NeuronCores.

Problem (per reference):
  q = wq @ x_q + bq ; k = wk @ x_kv + bk ; v = wv @ x_kv + bv   (1x1 convs)
  per head: attn = softmax(q^T k / sqrt(hd)) ; out = attn @ v^T
  y = wo @ out + bo

Sharding: core c -> (batch b = c // 4, head n = c % 4). Each core runs one
head's full attention and produces the partial output projection
y_part = wo[:, head] @ out_head; the host sums the 4 head partials per batch.

Device-side simplifications (all mathematically exact):
  * bk drops out entirely: a per-query constant shift of the logits cancels
    in softmax.
  * bv folds into the output bias: sum_j softmax_ij = 1, so v-bias
    contributes wo_col @ bv, added to bo on the host.
  * scale 1/8 folds into wq/bq on the host.
  * no max-subtraction: logits are ~N(0,1) (max |logit| < ~6), exp is safe
    in fp32.
  * softmax denominator comes from a ones-column appended to v^T in the AV
    matmul; normalization happens after AV on [64, S] instead of [S, S].

Layouts: logits are computed transposed, S^T[j, i] (k stationary, q moving),
so the exp'd tile feeds the AV matmul directly with j on partitions — no
transposes anywhere. v^T is produced directly by using x_kv chunks as the
stationary operand of the v projection.
"""

import numpy as np
import ml_dtypes

import concourse.bacc as bacc
import concourse.mybir as mybir
import concourse.tile as tile
from concourse.bass_utils import run_bass_kernel_spmd

F32 = mybir.dt.float32
BF16 = mybir.dt.bfloat16

B, C, HGT, WID = 2, 256, 64, 64
S = HGT * WID  # 4096 pixels
NH, HD = 4, 64
NCORES = 8
P = 128
IC = 1024  # i-chunk width (2 PSUM banks)
NI = S // IC  # 4
NJ = S // P  # 32 j-blocks
SCALE = HD ** -0.5
EXP_W = 1024  # free width of one exp instruction (must divide IC)


def _emit(tc):
    nc = tc.nc
    xq = nc.dram_tensor("xq", [2, P, S], BF16, kind="ExternalInput").ap()
    xkv = nc.dram_tensor("xkv", [2, P, S], BF16, kind="ExternalInput").ap()
    wqT = nc.dram_tensor("wqT", [2, P, HD], BF16, kind="ExternalInput").ap()
    wkT = nc.dram_tensor("wkT", [2, P, HD], BF16, kind="ExternalInput").ap()
    wvT = nc.dram_tensor("wvT", [2, P, HD], BF16, kind="ExternalInput").ap()
    woT = nc.dram_tensor("woT", [HD, C], BF16, kind="ExternalInput").ap()
    bq = nc.dram_tensor("bq", [HD, 1], F32, kind="ExternalInput").ap()
    bo = nc.dram_tensor("bo", [2, P, 1], F32, kind="ExternalInput").ap()
    y = nc.dram_tensor("y", [2, P, S], F32, kind="ExternalOutput").ap()

    with (
        tc.tile_pool(name="const", bufs=1) as cpool,
        tc.tile_pool(name="xp", bufs=1) as xpool,
        tc.tile_pool(name="qkv", bufs=1) as qpool,
        tc.tile_pool(name="es", bufs=3) as epool,
        tc.tile_pool(name="epi", bufs=2) as fpool,
        tc.tile_pool(name="ps", bufs=2, space="PSUM") as pp,
    ):
        # ---- weights / constants into SBUF ----
        wq_sb = cpool.tile([P, 2 * HD], BF16)
        wk_sb = cpool.tile([P, 2 * HD], BF16)
        wv_sb = cpool.tile([P, 2 * HD], BF16)
        for cch in range(2):
            nc.sync.dma_start(wq_sb[:, cch * HD:(cch + 1) * HD], wqT[cch])
            nc.sync.dma_start(wk_sb[:, cch * HD:(cch + 1) * HD], wkT[cch])
            nc.sync.dma_start(wv_sb[:, cch * HD:(cch + 1) * HD], wvT[cch])
        wo_sb = cpool.tile([HD, C], BF16)
        nc.sync.dma_start(wo_sb[:], woT)
        bq_sb = cpool.tile([HD, 1], F32)
        nc.sync.dma_start(bq_sb[:], bq)
        bo_sb = cpool.tile([P, 2], F32)
        for oh in range(2):
            nc.sync.dma_start(bo_sb[:, oh:oh + 1], bo[oh])
        ones_sb = cpool.tile([P, HD], BF16)
        nc.vector.memset(ones_sb[:], 1.0)

        # ---- activations into SBUF ----
        xq_sb = [xpool.tile([P, S], BF16, tag=f"xq{i}", name=f"xq_sb{i}")
                 for i in range(2)]
        xkv_sb = [xpool.tile([P, S], BF16, tag=f"xkv{i}", name=f"xkv_sb{i}")
                  for i in range(2)]
        for cch in range(2):
            nc.sync.dma_start(xq_sb[cch][:], xq[cch])
            nc.sync.dma_start(xkv_sb[cch][:], xkv[cch])

        q_sb = qpool.tile([HD, S], BF16)
        k_sb = qpool.tile([HD, S], BF16)
        # v^T with a ones column appended: [j-block partitions, (block, hd+1)]
        va_sb = qpool.tile([P, NJ * (HD + 1)], BF16)
        nc.vector.memset(va_sb[:], 1.0)

        # ---- q / k projections: [hd, S] = w^T.T @ x ----
        for t in range(S // 512):
            sl = slice(t * 512, (t + 1) * 512)
            qp = pp.tile([HD, 512], F32, tag="s")
            nc.tensor.matmul(qp[:], wq_sb[:, 0:HD], xq_sb[0][:, sl],
                             start=True, stop=False)
            nc.tensor.matmul(qp[:], wq_sb[:, HD:2 * HD], xq_sb[1][:, sl],
                             start=False, stop=True)
            nc.vector.tensor_scalar_add(q_sb[:, sl], qp[:], bq_sb[:])

            kp = pp.tile([HD, 512], F32, tag="s")
            nc.tensor.matmul(kp[:], wk_sb[:, 0:HD], xkv_sb[0][:, sl],
                             start=True, stop=False)
            nc.tensor.matmul(kp[:], wk_sb[:, HD:2 * HD], xkv_sb[1][:, sl],
                             start=False, stop=True)
            nc.vector.tensor_copy(k_sb[:, sl], kp[:])

        # ---- v^T projection: [j, hd] = x_kv(chunk).T @ wv^T(chunk) ----
        for j in range(NJ):
            jb = slice(j * P, (j + 1) * P)
            vp = pp.tile([P, HD], F32, tag="s")
            nc.tensor.matmul(vp[:], xkv_sb[0][:, jb], wv_sb[:, 0:HD],
                             start=True, stop=False)
            nc.tensor.matmul(vp[:], xkv_sb[1][:, jb], wv_sb[:, HD:2 * HD],
                             start=False, stop=True)
            base = j * (HD + 1)
            nc.vector.tensor_copy(va_sb[:, base:base + HD], vp[:])

        # ---- attention, i-chunk at a time ----
        for i in range(NI):
            av = pp.tile([HD + 1, IC], F32, tag="av")
            for j in range(NJ):
                jb = slice(j * P, (j + 1) * P)
                st = pp.tile([P, IC], F32, tag="s")
                for h in range(IC // 512):
                    isl = slice(i * IC + h * 512, i * IC + (h + 1) * 512)
                    nc.tensor.matmul(st[:, h * 512:(h + 1) * 512],
                                     k_sb[:, jb], q_sb[:, isl],
                                     start=True, stop=True)
                et = epool.tile([P, IC], BF16)
                for h in range(IC // EXP_W):
                    esl = slice(h * EXP_W, (h + 1) * EXP_W)
                    nc.scalar.activation(et[:, esl], st[:, esl],
                                         mybir.ActivationFunctionType.Exp)
                vbase = j * (HD + 1)
                for h in range(IC // 512):
                    nc.tensor.matmul(av[:, h * 512:(h + 1) * 512],
                                     va_sb[:, vbase:vbase + HD + 1],
                                     et[:, h * 512:(h + 1) * 512],
                                     start=(j == 0), stop=(j == NJ - 1))

            # ---- epilogue: normalize and project out ----
            rcp = fpool.tile([HD + 1, IC], F32)
            nc.vector.reciprocal(rcp[HD:HD + 1, :], av[HD:HD + 1, :])
            rcpb = fpool.tile([HD + 1, IC], BF16)
            nc.vector.tensor_copy(rcpb[HD:HD + 1, :], rcp[HD:HD + 1, :])
            bcm = pp.tile([HD, IC], F32, tag="s")
            for h in range(IC // 512):
                nc.tensor.matmul(bcm[:, h * 512:(h + 1) * 512],
                                 ones_sb[HD:HD + 1, :],
                                 rcpb[HD:HD + 1, h * 512:(h + 1) * 512],
                                 start=True, stop=True)
            rcq = fpool.tile([HD, IC], F32)
            nc.vector.tensor_copy(rcq[:], bcm[:])
            outt = fpool.tile([HD, IC], BF16)
            nc.vector.tensor_mul(outt[:], av[0:HD, :], rcq[:])

            for oh in range(2):
                for h in range(IC // 512):
                    yp = pp.tile([P, 512], F32, tag="s")
                    nc.tensor.matmul(yp[:], wo_sb[:, oh * P:(oh + 1) * P],
                                     outt[:, h * 512:(h + 1) * 512],
                                     start=True, stop=True)
                    ys = fpool.tile([P, 512], F32)
                    nc.vector.tensor_scalar_add(ys[:], yp[:],
                                                bo_sb[:, oh:oh + 1])
                    nc.sync.dma_start(
                        y[oh][:, i * IC + h * 512:i * IC + (h + 1) * 512],
                        ys[:])


def build():
    nc = bacc.Bacc("TRN2", target_bir_lowering=False, debug=False,
                   enable_asserts=False)
    with tile.TileContext(nc) as tc:
        _emit(tc)
    nc.compile()
    return nc


_NC_CACHE = []


def _get_nc():
    if not _NC_CACHE:
        _NC_CACHE.append(build())
    return _NC_CACHE[0]


def make_in_maps(x_q, x_kv, wq, bq, wk, bk, wv, bv, wo, bo):
    bf = ml_dtypes.bfloat16
    in_maps = []
    for c in range(NCORES):
        b, n = divmod(c, NH)
        hs = slice(n * HD, (n + 1) * HD)
        wq_h = wq[hs].astype(np.float64) * SCALE
        bo_eff = wo[:, hs].astype(np.float64) @ bv[hs].astype(np.float64)
        if n == 0:
            bo_eff = bo_eff + bo.astype(np.float64)
        in_maps.append({
            "xq": np.ascontiguousarray(
                x_q[b].reshape(C, S).reshape(2, P, S)).astype(bf),
            "xkv": np.ascontiguousarray(
                x_kv[b].reshape(C, S).reshape(2, P, S)).astype(bf),
            "wqT": np.ascontiguousarray(wq_h.T.reshape(2, P, HD)).astype(bf),
            "wkT": np.ascontiguousarray(
                wk[hs].T.reshape(2, P, HD)).astype(bf),
            "wvT": np.ascontiguousarray(
                wv[hs].T.reshape(2, P, HD)).astype(bf),
            "woT": np.ascontiguousarray(wo[:, hs].T).astype(bf),
            "bq": (bq[hs].astype(np.float64) * SCALE
                   ).astype(np.float32).reshape(HD, 1),
            "bo": bo_eff.astype(np.float32).reshape(2, P, 1),
        })
    return in_maps


def assemble_output(results):
    y = np.zeros((B, C, S), np.float32)
    for c in range(NCORES):
        b = c // NH
        y[b] += results[c]["y"].reshape(C, S).astype(np.float32)
    return y.reshape(B, C, HGT, WID)


def kernel(**inputs):
    nc = _get_nc()
    in_maps = make_in_maps(**inputs)
    res = run_bass_kernel_spmd(nc, in_maps, list(range(NCORES)))
    return assemble_output(res.results)


if __name__ == "__main__":
    nc = build()
    print("built + compiled ok")
